# revision 5
# baseline (speedup 1.0000x reference)
"""Trainium2 Bass kernel for nn_MemKDMClassModel (retrieval_knn).

Computation (per sample b, fully data-parallel over the batch):
    d2[b,i]   = ||x_enc[b] - x_neigh[b,i]||^2
    w[b,i]    = exp(-d2[b,i] / sigma^2)          (= k^2 with k the RBF kernel)
    probs[b,c]= sum_i w[b,i]*onehot(y[b,i])[c] / (sum_i w[b,i] + EPS)

Sharding: pure data parallel — batch split across 8 NeuronCores.

Per-core mapping (512 samples/core, blocks of 128 samples on partitions):
  - PE:  diff = I@neigh + I@(-x)  (PSUM accumulate) -> per-comp diff tile
  - ACT: Square activation with accum_out -> d2 column per comp
  - ACT: exp(scale * d2) with per-partition scale = -1/sigma^2
  - DVE: scatter probs += (iota == y_i) * w_i  (fused tensor_scalar + add)
"""

import numpy as np

BS, N_COMP, ENC, DIM_Y = 4096, 128, 512, 100
EPS = 1e-10
N_CORES = 8
BS_L = BS // N_CORES          # 512 samples per core
BLK = 128                     # samples per block (partition dim)
NBLK = BS_L // BLK            # 4 blocks per core
G = 8                         # comps per DMA transfer (2 MiB each)
NG = N_COMP // G              # 16 DMA groups per block

_CACHE: dict = {}


def _build_nc():
    import concourse.bacc as bacc
    import concourse.tile as tile
    import concourse.mybir as mybir
    from concourse import bass

    f32 = mybir.dt.float32
    i32 = mybir.dt.int32
    AF = mybir.ActivationFunctionType
    ALU = mybir.AluOpType
    AX = mybir.AxisListType

    nc = bacc.Bacc("TRN2", target_bir_lowering=False, debug=False,
                   num_devices=N_CORES)

    x_dram = nc.dram_tensor("x_enc", [BS_L, ENC], f32, kind="ExternalInput")
    n_dram = nc.dram_tensor("x_neigh", [BS_L, N_COMP, ENC], f32,
                            kind="ExternalInput")
    s_dram = nc.dram_tensor("sigma", [1, 1], f32, kind="ExternalInput")
    y_dram = nc.dram_tensor("y_neigh", [BS_L, N_COMP], i32,
                            kind="ExternalInput")
    eye_dram = nc.dram_tensor("eye", [128, 128], f32, kind="ExternalInput")
    iota_dram = nc.dram_tensor("iota", [128, DIM_Y], f32,
                               kind="ExternalInput")
    out_dram = nc.dram_tensor("out", [BS_L, DIM_Y], f32,
                              kind="ExternalOutput")

    with tile.TileContext(nc) as tc:
        with (
            tc.tile_pool(name="const", bufs=1) as constp,
            tc.tile_pool(name="neigh", bufs=3) as neighp,
            tc.tile_pool(name="xp", bufs=2) as xp,
            tc.tile_pool(name="small", bufs=2) as smallp,
            tc.tile_pool(name="ohp", bufs=4) as ohp,
            tc.tile_pool(name="outp", bufs=2) as outp,
            tc.tile_pool(name="pdiff", bufs=4, space=bass.MemorySpace.PSUM) as pdiff,
            tc.tile_pool(name="pscratch", bufs=1, space=bass.MemorySpace.PSUM) as pscratch,
            tc.tile_pool(name="pmisc", bufs=1, space=bass.MemorySpace.PSUM) as pmisc,
        ):
            # ---- constants ----
            eye = constp.tile([128, 128], f32)
            nc.sync.dma_start(eye[:], eye_dram[:])
            iota = constp.tile([128, DIM_Y], f32)
            nc.sync.dma_start(iota[:], iota_dram[:])

            # ---- cvec = -1/sigma^2 broadcast to [128, 1] ----
            sig = constp.tile([1, 1], f32)
            nc.sync.dma_start(sig[:], s_dram[:])
            sig2 = constp.tile([1, 1], f32)
            nc.vector.tensor_scalar(sig2[:], sig[:], sig[0:1, 0:1], None,
                                    op0=ALU.mult)
            rsig2 = constp.tile([1, 1], f32)
            nc.vector.reciprocal(rsig2[:], sig2[:])
            nrsig2 = constp.tile([1, 1], f32)
            nc.vector.tensor_scalar_mul(nrsig2[:], rsig2[:], -1.0)
            ones_row = constp.tile([1, 128], f32)
            nc.vector.memset(ones_row[:], 1.0)
            cvec_ps = pmisc.tile([128, 1], f32)
            nc.tensor.matmul(cvec_ps[:], ones_row[:], nrsig2[:],
                             start=True, stop=True)
            cvec = constp.tile([128, 1], f32)
            nc.vector.tensor_copy(cvec[:], cvec_ps[:])

            sq_scratch = pscratch.tile([128, ENC], f32)

            for b in range(NBLK):
                s0 = b * BLK
                # ---- per-block inputs ----
                x_tile = xp.tile([BLK, ENC], f32, tag="x")
                nc.sync.dma_start(x_tile[:], x_dram[s0:s0 + BLK, :])
                negx = xp.tile([BLK, ENC], f32, tag="negx")
                nc.scalar.mul(negx[:], x_tile[:], -1.0)

                y_tile = smallp.tile([BLK, N_COMP], i32, tag="y")
                nc.sync.dma_start(y_tile[:], y_dram[s0:s0 + BLK, :])
                y_f32 = smallp.tile([BLK, N_COMP], f32, tag="yf")
                nc.vector.tensor_copy(y_f32[:], y_tile[:])

                d2 = smallp.tile([BLK, N_COMP], f32, tag="d2")

                # ---- main stream: d2 columns ----
                for g in range(NG):
                    ntile = neighp.tile([BLK, G * ENC], f32)
                    nc.sync.dma_start(
                        ntile[:], n_dram[s0:s0 + BLK, g * G:(g + 1) * G, :])
                    for j in range(G):
                        i = g * G + j
                        dtile = pdiff.tile([BLK, ENC], f32)
                        nc.tensor.matmul(dtile[:], eye[:],
                                         ntile[:, j * ENC:(j + 1) * ENC],
                                         start=True, stop=False)
                        nc.tensor.matmul(dtile[:], eye[:], negx[:],
                                         start=False, stop=True)
                        nc.scalar.activation(sq_scratch[:], dtile[:],
                                             AF.Square,
                                             accum_out=d2[:, i:i + 1])

                # ---- epilogue ----
                w = smallp.tile([BLK, N_COMP], f32, tag="w")
                nc.scalar.activation(w[:], d2[:], AF.Exp,
                                     scale=cvec[:, 0:1])
                rowsum = smallp.tile([BLK, 1], f32, tag="rs")
                nc.vector.reduce_sum(rowsum[:], w[:], axis=AX.X)
                rs_eps = smallp.tile([BLK, 1], f32, tag="rse")
                nc.vector.tensor_scalar_add(rs_eps[:], rowsum[:], EPS)
                rinv = smallp.tile([BLK, 1], f32, tag="rinv")
                nc.vector.reciprocal(rinv[:], rs_eps[:])

                # ---- scatter: probs += (iota == y_i) * w_i ----
                probs = outp.tile([BLK, DIM_Y], f32, tag="probs")
                nc.vector.tensor_scalar(probs[:], iota[:],
                                        y_f32[:, 0:1], w[:, 0:1],
                                        op0=ALU.is_equal, op1=ALU.mult)
                for i in range(1, N_COMP):
                    oh = ohp.tile([BLK, DIM_Y], f32, tag="oh")
                    nc.vector.tensor_scalar(oh[:], iota[:],
                                            y_f32[:, i:i + 1], w[:, i:i + 1],
                                            op0=ALU.is_equal, op1=ALU.mult)
                    nc.vector.tensor_tensor(probs[:], probs[:], oh[:],
                                            op=ALU.add)

                out_sb = outp.tile([BLK, DIM_Y], f32, tag="out")
                nc.vector.tensor_scalar(out_sb[:], probs[:],
                                        rinv[:, 0:1], None, op0=ALU.mult)
                nc.sync.dma_start(out_dram[s0:s0 + BLK, :], out_sb[:])

    nc.compile()
    return nc


def _get_nc():
    if "nc" not in _CACHE:
        _CACHE["nc"] = _build_nc()
    return _CACHE["nc"]


def _get_exec():
    """Build (once) a jitted shard_map executable over 8 cores.

    Returns (fn, in_names, out_names, out_avals, n_params, mesh).
    Call as fn(*concat_inputs, *concat_zero_outputs); outputs donated.
    """
    if "exec" in _CACHE:
        return _CACHE["exec"]
    import jax
    import concourse.mybir as mybir
    from jax.sharding import Mesh, PartitionSpec
    from jax.experimental.shard_map import shard_map
    from concourse.bass2jax import (_bass_exec_p, install_neuronx_cc_hook,
                                    partition_id_tensor)

    install_neuronx_cc_hook()
    nc = _get_nc()
    partition_name = (nc.partition_id_tensor.name
                      if nc.partition_id_tensor else None)
    in_names, out_names, out_avals = [], [], []
    for alloc in nc.m.functions[0].allocations:
        if not isinstance(alloc, mybir.MemoryLocationSet):
            continue
        name = alloc.memorylocations[0].name
        if alloc.kind == "ExternalInput":
            if name != partition_name:
                in_names.append(name)
        elif alloc.kind == "ExternalOutput":
            out_names.append(name)
            out_avals.append(jax.core.ShapedArray(
                tuple(alloc.tensor_shape), mybir.dt.np(alloc.dtype)))
    n_params = len(in_names)
    all_in_names = in_names + out_names
    if partition_name is not None:
        all_in_names = all_in_names + [partition_name]
    donate = tuple(range(n_params, n_params + len(out_names)))

    def _body(*args):
        operands = list(args)
        if partition_name is not None:
            operands.append(partition_id_tensor())
        outs = _bass_exec_p.bind(
            *operands,
            out_avals=tuple(out_avals),
            in_names=tuple(all_in_names),
            out_names=tuple(out_names),
            lowering_input_output_aliases=(),
            sim_require_finite=True,
            sim_require_nnan=True,
            nc=nc,
        )
        return tuple(outs)

    devices = jax.devices()[:N_CORES]
    mesh = Mesh(np.asarray(devices), ("core",))
    specs = (PartitionSpec("core"),) * (n_params + len(out_names))
    out_specs = (PartitionSpec("core"),) * len(out_names)
    fn = jax.jit(
        shard_map(_body, mesh=mesh, in_specs=specs, out_specs=out_specs,
                  check_rep=False),
        donate_argnums=donate, keep_unused=True)
    _CACHE["exec"] = (fn, in_names, out_names, out_avals, n_params, mesh)
    return _CACHE["exec"]


def _concat_inputs(x_enc, x_neigh, sig, y_neigh_i32):
    """Per-input concatenation over cores, ordered by the NEFF's in_names."""
    eye = np.eye(128, dtype=np.float32)
    iota = np.broadcast_to(np.arange(DIM_Y, dtype=np.float32),
                           (128, DIM_Y)).copy()
    per_core = {
        "x_enc": lambda c: x_enc[c * BS_L:(c + 1) * BS_L],
        "x_neigh": lambda c: x_neigh[c * BS_L:(c + 1) * BS_L],
        "sigma": lambda c: sig,
        "y_neigh": lambda c: y_neigh_i32[c * BS_L:(c + 1) * BS_L],
        "eye": lambda c: eye,
        "iota": lambda c: iota,
    }
    _, in_names, _, _, _, _ = _get_exec()
    return [np.concatenate([per_core[name](c) for c in range(N_CORES)], axis=0)
            for name in in_names]


def _zero_outs():
    _, _, _, out_avals, _, _ = _get_exec()
    return [np.zeros((N_CORES * a.shape[0], *a.shape[1:]), a.dtype)
            for a in out_avals]


def kernel(x_enc, x_neigh, sigma, y_neigh):
    x_enc = np.ascontiguousarray(np.asarray(x_enc, dtype=np.float32))
    x_neigh = np.ascontiguousarray(np.asarray(x_neigh, dtype=np.float32))
    sig = np.ascontiguousarray(np.asarray(sigma).astype(np.float32).reshape(1, 1))
    y_neigh_i32 = np.ascontiguousarray(np.asarray(y_neigh).astype(np.int32))

    fn, in_names, out_names, out_avals, n_params, mesh = _get_exec()
    concat_in = _concat_inputs(x_enc, x_neigh, sig, y_neigh_i32)
    out_arrs = fn(*concat_in, *_zero_outs())
    oi = out_names.index("out")
    out = np.asarray(out_arrs[oi]).reshape(N_CORES, BS_L, DIM_Y)
    return out.reshape(BS, DIM_Y).astype(np.float32)


if __name__ == "__main__":
    rng = np.random.default_rng(0)
    x_enc = rng.standard_normal((BS, ENC), dtype=np.float32)
    x_neigh = rng.standard_normal((BS, N_COMP, ENC), dtype=np.float32)
    sigma = 20.0 * np.ones((1,), dtype=np.float32)  # large: exercises nonzero path
    y_neigh = rng.integers(0, DIM_Y, size=(BS, N_COMP)).astype(np.int32)
    out = kernel(x_enc=x_enc, x_neigh=x_neigh, sigma=sigma, y_neigh=y_neigh)
    # numpy oracle
    d2 = np.maximum(
        (x_enc ** 2).sum(-1)[:, None]
        + (x_neigh ** 2).sum(-1)
        - 2.0 * np.einsum("bd,bnd->bn", x_enc, x_neigh), 0.0)
    w = np.exp(-d2 / (sigma[0] ** 2))
    probs = np.zeros((BS, DIM_Y), np.float32)
    np.add.at(probs, (np.arange(BS)[:, None], y_neigh), w.astype(np.float32))
    probs /= (w.sum(-1, keepdims=True).astype(np.float32) + EPS)
    print("max abs diff:", np.abs(out - probs).max())
    print("ref max:", probs.max(), "out max:", out.max())
    print("out nonzero:", np.count_nonzero(out), "/", out.size)


# revision 20
# speedup vs baseline: 2.1868x; 2.1868x over previous
"""Trainium2 Bass kernel for nn_MemKDMClassModel (retrieval_knn).

Computation (per sample b, fully data-parallel over the batch):
    d2[b,i]   = ||x_enc[b] - x_neigh[b,i]||^2
    w[b,i]    = exp(-d2[b,i] / sigma^2)          (= k^2 with k the RBF kernel)
    probs[b,c]= sum_i w[b,i]*onehot(y[b,i])[c] / (sum_i w[b,i] + EPS)

Sharding: pure data parallel — batch split across 8 NeuronCores.

Per-core mapping (512 samples/core, blocks of 128 samples on partitions):
  - PE:  diff = I@neigh + I@(-x)  (PSUM accumulate) -> per-comp diff tile
  - ACT: Square activation with accum_out -> d2 column per comp
  - ACT: exp(scale * d2) with per-partition scale = -1/sigma^2
  - DVE: scatter probs += (iota == y_i) * w_i  (fused tensor_scalar + add)
"""

import numpy as np

BS, N_COMP, ENC, DIM_Y = 4096, 128, 512, 100
EPS = 1e-10
N_CORES = 8
BS_L = BS // N_CORES          # 512 samples per core
BLK = 128                     # samples per block (partition dim)
NBLK = BS_L // BLK            # 4 blocks per core
G = 8                         # comps per DMA transfer (2 MiB each)
NG = N_COMP // G              # 16 DMA groups per block

# Per-block comp split across engine paths (load balance, see analyze.py):
#   P: PE fp32 diff matmuls -> ACT Square accum      (PE + ACT)
#   M: DVE STT -2*x.n       -> ACT Square accum n2   (DVE + ACT)
#   V: DVE STT -2*x.n       -> DVE STT n2            (DVE only)
NP, NM, NV = 53, 61, 14
assert NP + NM + NV == N_COMP


CH = 8                 # epilogue chunks per block (pipelined into the stream)
CW = N_COMP // CH      # columns per chunk


def _build_paths():
    """Per chunk of CW comps (arrival order): interleave P/M/V inside the
    chunk; map comps to d2 columns so each chunk's P columns come first,
    then its M/V columns. Returns per-comp path/col, col->comp, and
    per-chunk (n_p, n_mv)."""
    base = {"P": NP, "M": NM, "V": NV}
    paths = [None] * N_COMP
    col_of = [0] * N_COMP
    comp_of = [0] * N_COMP
    chunk_np = []
    for c in range(CH):
        take = {k: (base[k] * (c + 1)) // CH - (base[k] * c) // CH
                for k in ("P", "M")}
        take["V"] = CW - take["P"] - take["M"]
        assert take["V"] >= 0
        chunk_np.append(take["P"])
        acc = {k: 0 for k in take}
        order = []
        for t in range(CW):
            avail = [q for q in take if acc[q] < take[q]]
            k = max(avail, key=lambda q: take[q] * (t + 1) / CW - acc[q])
            order.append(k)
            acc[k] += 1
        pcol = c * CW
        mvcol = c * CW + take["P"]
        for t, k in enumerate(order):
            i = c * CW + t
            paths[i] = k
            if k == "P":
                col = pcol
                pcol += 1
            else:
                col = mvcol
                mvcol += 1
            col_of[i] = col
            comp_of[col] = i
    return paths, col_of, comp_of, chunk_np


PATHS, COL_OF, COMP_OF, CHUNK_NP = _build_paths()

_CACHE: dict = {}


def _build_nc():
    import concourse.bacc as bacc
    import concourse.tile as tile
    import concourse.mybir as mybir
    from concourse import bass

    f32 = mybir.dt.float32
    i32 = mybir.dt.int32
    AF = mybir.ActivationFunctionType
    ALU = mybir.AluOpType
    AX = mybir.AxisListType

    nc = bacc.Bacc("TRN2", target_bir_lowering=False, debug=False,
                   num_devices=N_CORES)

    x_dram = nc.dram_tensor("x_enc", [BS_L, ENC], f32, kind="ExternalInput")
    n_dram = nc.dram_tensor("x_neigh", [BS_L, N_COMP, ENC], f32,
                            kind="ExternalInput")
    s_dram = nc.dram_tensor("sigma", [1, 1], f32, kind="ExternalInput")
    y_dram = nc.dram_tensor("y_neigh", [BS_L, N_COMP], i32,
                            kind="ExternalInput")
    eye_dram = nc.dram_tensor("eye", [128, 128], f32, kind="ExternalInput")
    iota_dram = nc.dram_tensor("iota", [128, DIM_Y], f32,
                               kind="ExternalInput")
    out_dram = nc.dram_tensor("out", [BS_L, DIM_Y], f32,
                              kind="ExternalOutput")

    with tile.TileContext(nc) as tc:
        with (
            tc.tile_pool(name="const", bufs=1) as constp,
            tc.tile_pool(name="neigh", bufs=8) as neighp,
            tc.tile_pool(name="xp", bufs=3) as xp,
            tc.tile_pool(name="small", bufs=3) as smallp,
            tc.tile_pool(name="ohp", bufs=8) as ohp,
            tc.tile_pool(name="outp", bufs=3) as outp,
            tc.tile_pool(name="pdiff", bufs=6, space=bass.MemorySpace.PSUM) as pdiff,
            tc.tile_pool(name="pscratch", bufs=1, space=bass.MemorySpace.PSUM) as pscratch,
            tc.tile_pool(name="pmisc", bufs=1, space=bass.MemorySpace.PSUM) as pmisc,
        ):
            # ---- constants ----
            eye = constp.tile([128, 128], f32)
            nc.sync.dma_start(eye[:], eye_dram[:])
            iota = constp.tile([128, DIM_Y], f32)
            nc.sync.dma_start(iota[:], iota_dram[:])

            # ---- cvec = -1/sigma^2 broadcast to [128, 1] ----
            sig = constp.tile([1, 1], f32)
            nc.sync.dma_start(sig[:], s_dram[:])
            sig2 = constp.tile([1, 1], f32)
            nc.vector.tensor_scalar(sig2[:], sig[:], sig[0:1, 0:1], None,
                                    op0=ALU.mult)
            rsig2 = constp.tile([1, 1], f32)
            nc.vector.reciprocal(rsig2[:], sig2[:])
            nrsig2 = constp.tile([1, 1], f32)
            nc.vector.tensor_scalar_mul(nrsig2[:], rsig2[:], -1.0)
            ones_row = constp.tile([1, 128], f32)
            nc.vector.memset(ones_row[:], 1.0)
            cvec_ps = pmisc.tile([128, 1], f32)
            nc.tensor.matmul(cvec_ps[:], ones_row[:], nrsig2[:],
                             start=True, stop=True)
            cvec = constp.tile([128, 1], f32)
            nc.vector.tensor_copy(cvec[:], cvec_ps[:])

            sq_scratch = pscratch.tile([128, ENC], f32)
            ttr_scratch = constp.tile([128, ENC], f32)

            for b in range(NBLK):
                s0 = b * BLK
                # ---- per-block inputs ----
                x_tile = xp.tile([BLK, ENC], f32, tag="x")
                nc.sync.dma_start(x_tile[:], x_dram[s0:s0 + BLK, :])
                negx = xp.tile([BLK, ENC], f32, tag="negx")
                nc.vector.tensor_scalar_mul(negx[:], x_tile[:], -1.0)
                x2col = smallp.tile([BLK, 1], f32, tag="x2")
                nc.scalar.activation(sq_scratch[:], x_tile[:], AF.Square,
                                     accum_out=x2col[:, 0:1])

                y_tile = smallp.tile([BLK, N_COMP], i32, tag="y")
                nc.sync.dma_start(y_tile[:], y_dram[s0:s0 + BLK, :])
                y_f32 = smallp.tile([BLK, N_COMP], f32, tag="yf")
                nc.vector.tensor_copy(y_f32[:], y_tile[:])

                d2 = smallp.tile([BLK, N_COMP], f32, tag="d2")
                w = smallp.tile([BLK, N_COMP], f32, tag="w")
                probs = outp.tile([BLK, DIM_Y], f32, tag="probs")
                rowsum = smallp.tile([BLK, 1], f32, tag="rs")

                # ---- main stream, chunked: stream + pipelined epilogue ----
                GPC = NG // CH   # DMA groups per chunk
                for c in range(CH):
                    np_c = CHUNK_NP[c]
                    mv0 = c * CW + np_c         # first MV column of chunk
                    mv1 = (c + 1) * CW
                    nmv = mv1 - mv0
                    t_ch = smallp.tile([BLK, nmv], f32, tag="tmv")
                    n2_ch = smallp.tile([BLK, nmv], f32, tag="n2mv")
                    for g in range(c * GPC, (c + 1) * GPC):
                        ntile = neighp.tile([BLK, G * ENC], f32, tag="ntile")
                        nc.sync.dma_start(
                            ntile[:],
                            n_dram[s0:s0 + BLK, g * G:(g + 1) * G, :])
                        for j in range(G):
                            i = g * G + j
                            nsl = ntile[:, j * ENC:(j + 1) * ENC]
                            path, col = PATHS[i], COL_OF[i]
                            if path == "P":
                                # P: PE diff -> ACT square accum (true d2)
                                dtile = pdiff.tile([BLK, ENC], f32)
                                nc.tensor.matmul(dtile[:], eye[:], nsl,
                                                 start=True, stop=False)
                                nc.tensor.matmul(dtile[:], eye[:], negx[:],
                                                 start=False, stop=True)
                                nc.scalar.activation(
                                    sq_scratch[:], dtile[:], AF.Square,
                                    accum_out=d2[:, col:col + 1])
                            else:
                                k = col - mv0
                                # t = -2*sum(x*n) (fused mul+reduce on DVE)
                                nc.vector.scalar_tensor_tensor(
                                    ttr_scratch[:], nsl, -2.0, x_tile[:],
                                    op0=ALU.mult, op1=ALU.mult,
                                    accum_out=t_ch[:, k:k + 1])
                                if path == "M":
                                    # M: n2 on ACT
                                    nc.scalar.activation(
                                        sq_scratch[:], nsl, AF.Square,
                                        accum_out=n2_ch[:, k:k + 1])
                                else:
                                    # V: n2 on DVE
                                    nc.vector.scalar_tensor_tensor(
                                        ttr_scratch[:], nsl, 1.0, nsl,
                                        op0=ALU.mult, op1=ALU.mult,
                                        accum_out=n2_ch[:, k:k + 1])

                    # ---- chunk epilogue (overlaps next chunk's stream) ----
                    # d2[MV cols] = max(t + n2 + x2, 0)
                    nc.vector.tensor_tensor(d2[:, mv0:mv1], t_ch[:],
                                            n2_ch[:], op=ALU.add)
                    nc.vector.tensor_scalar(d2[:, mv0:mv1], d2[:, mv0:mv1],
                                            x2col[:, 0:1], 0.0,
                                            op0=ALU.add, op1=ALU.max)
                    # w = exp(-d2/sigma^2) for this chunk's columns
                    lo, hi = c * CW, (c + 1) * CW
                    nc.scalar.activation(w[:, lo:hi], d2[:, lo:hi], AF.Exp,
                                         scale=cvec[:, 0:1])
                    if c == 0:
                        nc.vector.reduce_sum(rowsum[:], w[:, lo:hi],
                                             axis=AX.X)
                    else:
                        rs_ch = smallp.tile([BLK, 1], f32, tag="rsch")
                        nc.vector.reduce_sum(rs_ch[:], w[:, lo:hi],
                                             axis=AX.X)
                        nc.vector.tensor_tensor(rowsum[:], rowsum[:],
                                                rs_ch[:], op=ALU.add)
                    # scatter this chunk: probs += (iota == y_ci) * w_col
                    for col in range(lo, hi):
                        ci = COMP_OF[col]
                        if col == 0:
                            nc.vector.tensor_scalar(
                                probs[:], iota[:], y_f32[:, ci:ci + 1],
                                w[:, 0:1], op0=ALU.is_equal, op1=ALU.mult)
                            continue
                        oh = ohp.tile([BLK, DIM_Y], f32, tag="oh")
                        nc.vector.tensor_scalar(oh[:], iota[:],
                                                y_f32[:, ci:ci + 1],
                                                w[:, col:col + 1],
                                                op0=ALU.is_equal,
                                                op1=ALU.mult)
                        nc.vector.tensor_tensor(probs[:], probs[:], oh[:],
                                                op=ALU.add)

                # ---- block tail: normalize + store ----
                rs_eps = smallp.tile([BLK, 1], f32, tag="rse")
                nc.vector.tensor_scalar_add(rs_eps[:], rowsum[:], EPS)
                rinv = smallp.tile([BLK, 1], f32, tag="rinv")
                nc.vector.reciprocal(rinv[:], rs_eps[:])
                out_sb = outp.tile([BLK, DIM_Y], f32, tag="out")
                nc.vector.tensor_scalar(out_sb[:], probs[:],
                                        rinv[:, 0:1], None, op0=ALU.mult)
                nc.gpsimd.dma_start(out_dram[s0:s0 + BLK, :], out_sb[:])

    nc.compile()
    return nc


def _get_nc():
    if "nc" not in _CACHE:
        _CACHE["nc"] = _build_nc()
    return _CACHE["nc"]


def _get_exec():
    """Build (once) a jitted shard_map executable over 8 cores.

    Returns (fn, in_names, out_names, out_avals, n_params, mesh).
    Call as fn(*concat_inputs, *concat_zero_outputs); outputs donated.
    """
    if "exec" in _CACHE:
        return _CACHE["exec"]
    import jax
    import concourse.mybir as mybir
    from jax.sharding import Mesh, PartitionSpec
    from jax.experimental.shard_map import shard_map
    from concourse.bass2jax import (_bass_exec_p, install_neuronx_cc_hook,
                                    partition_id_tensor)

    install_neuronx_cc_hook()
    nc = _get_nc()
    partition_name = (nc.partition_id_tensor.name
                      if nc.partition_id_tensor else None)
    in_names, out_names, out_avals = [], [], []
    for alloc in nc.m.functions[0].allocations:
        if not isinstance(alloc, mybir.MemoryLocationSet):
            continue
        name = alloc.memorylocations[0].name
        if alloc.kind == "ExternalInput":
            if name != partition_name:
                in_names.append(name)
        elif alloc.kind == "ExternalOutput":
            out_names.append(name)
            out_avals.append(jax.core.ShapedArray(
                tuple(alloc.tensor_shape), mybir.dt.np(alloc.dtype)))
    n_params = len(in_names)
    all_in_names = in_names + out_names
    if partition_name is not None:
        all_in_names = all_in_names + [partition_name]
    donate = tuple(range(n_params, n_params + len(out_names)))

    def _body(*args):
        operands = list(args)
        if partition_name is not None:
            operands.append(partition_id_tensor())
        outs = _bass_exec_p.bind(
            *operands,
            out_avals=tuple(out_avals),
            in_names=tuple(all_in_names),
            out_names=tuple(out_names),
            lowering_input_output_aliases=(),
            sim_require_finite=True,
            sim_require_nnan=True,
            nc=nc,
        )
        return tuple(outs)

    devices = jax.devices()[:N_CORES]
    mesh = Mesh(np.asarray(devices), ("core",))
    specs = (PartitionSpec("core"),) * (n_params + len(out_names))
    out_specs = (PartitionSpec("core"),) * len(out_names)
    fn = jax.jit(
        shard_map(_body, mesh=mesh, in_specs=specs, out_specs=out_specs,
                  check_rep=False),
        donate_argnums=donate, keep_unused=True)
    _CACHE["exec"] = (fn, in_names, out_names, out_avals, n_params, mesh)
    return _CACHE["exec"]


def _concat_inputs(x_enc, x_neigh, sig, y_neigh_i32):
    """Per-input concatenation over cores, ordered by the NEFF's in_names."""
    eye = np.eye(128, dtype=np.float32)
    iota = np.broadcast_to(np.arange(DIM_Y, dtype=np.float32),
                           (128, DIM_Y)).copy()
    per_core = {
        "x_enc": lambda c: x_enc[c * BS_L:(c + 1) * BS_L],
        "x_neigh": lambda c: x_neigh[c * BS_L:(c + 1) * BS_L],
        "sigma": lambda c: sig,
        "y_neigh": lambda c: y_neigh_i32[c * BS_L:(c + 1) * BS_L],
        "eye": lambda c: eye,
        "iota": lambda c: iota,
    }
    _, in_names, _, _, _, _ = _get_exec()
    return [np.concatenate([per_core[name](c) for c in range(N_CORES)], axis=0)
            for name in in_names]


def _zero_outs():
    _, _, _, out_avals, _, _ = _get_exec()
    return [np.zeros((N_CORES * a.shape[0], *a.shape[1:]), a.dtype)
            for a in out_avals]


def kernel(x_enc, x_neigh, sigma, y_neigh):
    x_enc = np.ascontiguousarray(np.asarray(x_enc, dtype=np.float32))
    x_neigh = np.ascontiguousarray(np.asarray(x_neigh, dtype=np.float32))
    sig = np.ascontiguousarray(np.asarray(sigma).astype(np.float32).reshape(1, 1))
    y_neigh_i32 = np.ascontiguousarray(np.asarray(y_neigh).astype(np.int32))

    fn, in_names, out_names, out_avals, n_params, mesh = _get_exec()
    concat_in = _concat_inputs(x_enc, x_neigh, sig, y_neigh_i32)
    out_arrs = fn(*concat_in, *_zero_outs())
    oi = out_names.index("out")
    out = np.asarray(out_arrs[oi]).reshape(N_CORES, BS_L, DIM_Y)
    return out.reshape(BS, DIM_Y).astype(np.float32)


if __name__ == "__main__":
    rng = np.random.default_rng(0)
    x_enc = rng.standard_normal((BS, ENC), dtype=np.float32)
    x_neigh = rng.standard_normal((BS, N_COMP, ENC), dtype=np.float32)
    sigma = 20.0 * np.ones((1,), dtype=np.float32)  # large: exercises nonzero path
    y_neigh = rng.integers(0, DIM_Y, size=(BS, N_COMP)).astype(np.int32)
    out = kernel(x_enc=x_enc, x_neigh=x_neigh, sigma=sigma, y_neigh=y_neigh)
    # numpy oracle
    d2 = np.maximum(
        (x_enc ** 2).sum(-1)[:, None]
        + (x_neigh ** 2).sum(-1)
        - 2.0 * np.einsum("bd,bnd->bn", x_enc, x_neigh), 0.0)
    w = np.exp(-d2 / (sigma[0] ** 2))
    probs = np.zeros((BS, DIM_Y), np.float32)
    np.add.at(probs, (np.arange(BS)[:, None], y_neigh), w.astype(np.float32))
    probs /= (w.sum(-1, keepdims=True).astype(np.float32) + EPS)
    print("max abs diff:", np.abs(out - probs).max())
    print("ref max:", probs.max(), "out max:", out.max())
    print("out nonzero:", np.count_nonzero(out), "/", out.size)
